# revision 13
# baseline (speedup 1.0000x reference)
"""Trainium2 Bass kernel for nn_AE_spikes (spiking autoencoder, 16-step scan).

Data-parallel over 8 NeuronCores: batch 16384 -> 2048 rows/core.

Layout: feature-major ("transposed") on device. All [784]-row tensors are
stored as [128 partitions, 7*NT] with chunk c (feature rows 128c..128c+kc)
occupying columns [c*NT, (c+1)*NT). Batch tile NT columns.

Engine plan:
  PE    : all matmuls as fp16 hi/lo weight splits (exact products for binary
          spikes), hidden membranes v1..v3 PSUM-resident, accumulated by
          matmul only; spike resets via -I identity matmuls.
  DVE   : threshold compares (is_ge, exact fp32 semantics), v0/v4 updates,
          spike-count pair tree in fp16.
  GPSIMD: encoder integrate (v0 += f).
"""

import os
import sys

import numpy as np

if "/opt/trn_rl_repo" not in sys.path:
    sys.path.insert(0, "/opt/trn_rl_repo")

B = 16384
IN = 784
H = 128
T = 16
NCORES = 8
BC = B // NCORES          # 2048 batch rows per core
NT = 512                  # batch-tile columns
NTILES = BC // NT         # 4
CH = 7                    # feature chunks: 6*128 + 16
KC = [128] * 6 + [16]
CW = CH * NT              # concatenated width, 3584

LAST_RESULT = None
_CACHE = {}


def _install_ntff_shim():
    """Make run_bass_kernel_spmd(trace=True) work in this container."""
    import types

    try:
        from antenv.axon_hooks import get_axon_ntff_profile_hook  # noqa: F401
        return
    except ImportError:
        pass
    try:
        import antenv
        from trn_agent_boot.trn_boot import _ntff_profile_via_ctypes
    except ImportError:
        return
    mod = types.ModuleType("antenv.axon_hooks")
    mod._hook = _ntff_profile_via_ctypes("/opt/axon/libaxon_pjrt.so")
    mod.set_axon_ntff_profile_hook = lambda h: setattr(mod, "_hook", h)
    mod.get_axon_ntff_profile_hook = lambda: mod._hook
    sys.modules["antenv.axon_hooks"] = mod
    antenv.axon_hooks = mod


def _build():
    import concourse.tile as tile
    from concourse import bacc, mybir
    from contextlib import ExitStack

    f32 = mybir.dt.float32
    f16 = mybir.dt.float16
    Alu = mybir.AluOpType

    nc = bacc.Bacc("TRN2", target_bir_lowering=False, debug=False)

    fT_d = nc.dram_tensor("fT", [IN, BC], f32, kind="ExternalInput").ap()
    w1hi_d = nc.dram_tensor("w1hi", [IN, H], f16, kind="ExternalInput").ap()
    w1lo_d = nc.dram_tensor("w1lo", [IN, H], f16, kind="ExternalInput").ap()
    w2hi_d = nc.dram_tensor("w2hi", [H, H], f16, kind="ExternalInput").ap()
    w2lo_d = nc.dram_tensor("w2lo", [H, H], f16, kind="ExternalInput").ap()
    w3hi_d = nc.dram_tensor("w3hi", [H, H], f16, kind="ExternalInput").ap()
    w3lo_d = nc.dram_tensor("w3lo", [H, H], f16, kind="ExternalInput").ap()
    w4hi_d = nc.dram_tensor("w4hi", [H, IN], f16, kind="ExternalInput").ap()
    w4lo_d = nc.dram_tensor("w4lo", [H, IN], f16, kind="ExternalInput").ap()
    negI_d = nc.dram_tensor("negI", [H, H], f16, kind="ExternalInput").ap()
    thh_d = nc.dram_tensor("thh", [H, 3 * T], f32, kind="ExternalInput").ap()
    th4_d = nc.dram_tensor("th4", [H, CH * T], f32, kind="ExternalInput").ap()
    out_d = nc.dram_tensor("outT", [IN, BC], f32, kind="ExternalOutput").ap()

    with tile.TileContext(nc) as tc:
        with ExitStack() as ctx:
            wp = ctx.enter_context(tc.tile_pool(name="weights", bufs=1))
            fp = ctx.enter_context(tc.tile_pool(name="feat", bufs=4))
            vp0 = ctx.enter_context(tc.tile_pool(name="v0p", bufs=2))
            sp0 = ctx.enter_context(tc.tile_pool(name="s0p", bufs=4))
            vp4 = ctx.enter_context(tc.tile_pool(name="v4p", bufs=2))
            sp4 = ctx.enter_context(tc.tile_pool(name="s4p", bufs=4))
            shp = ctx.enter_context(tc.tile_pool(name="shid", bufs=3))
            cnp = ctx.enter_context(tc.tile_pool(name="cnt", bufs=2))
            outp = ctx.enter_context(tc.tile_pool(name="outp", bufs=1))
            psv = ctx.enter_context(tc.tile_pool(name="psv", bufs=1, space="PSUM"))
            psv4 = ctx.enter_context(tc.tile_pool(name="psv4", bufs=1, space="PSUM"))
            psp4 = ctx.enter_context(tc.tile_pool(name="psp4", bufs=2, space="PSUM"))

            # ---- load weights / tables once ----
            w1hi = []
            w1lo = []
            for c in range(CH):
                kc = KC[c]
                thi = wp.tile([kc, H], f16, tag=f"w1hi{c}")
                nc.sync.dma_start(thi[:], w1hi_d[128 * c : 128 * c + kc, :])
                tlo = wp.tile([kc, H], f16, tag=f"w1lo{c}")
                nc.sync.dma_start(tlo[:], w1lo_d[128 * c : 128 * c + kc, :])
                w1hi.append(thi)
                w1lo.append(tlo)
            w2hi = wp.tile([H, H], f16, tag="w2hi")
            nc.sync.dma_start(w2hi[:], w2hi_d[:])
            w2lo = wp.tile([H, H], f16, tag="w2lo")
            nc.sync.dma_start(w2lo[:], w2lo_d[:])
            w3hi = wp.tile([H, H], f16, tag="w3hi")
            nc.sync.dma_start(w3hi[:], w3hi_d[:])
            w3lo = wp.tile([H, H], f16, tag="w3lo")
            nc.sync.dma_start(w3lo[:], w3lo_d[:])
            w4hi = []
            w4lo = []
            for m in range(CH):
                mc = KC[m]
                thi = wp.tile([H, mc], f16, tag=f"w4hi{m}")
                nc.sync.dma_start(thi[:], w4hi_d[:, 128 * m : 128 * m + mc])
                tlo = wp.tile([H, mc], f16, tag=f"w4lo{m}")
                nc.sync.dma_start(tlo[:], w4lo_d[:, 128 * m : 128 * m + mc])
                w4hi.append(thi)
                w4lo.append(tlo)
            negI = wp.tile([H, H], f16, tag="negI")
            nc.sync.dma_start(negI[:], negI_d[:])
            thh = wp.tile([H, 3 * T], f32, tag="thh")
            nc.sync.dma_start(thh[:], thh_d[:])
            th4 = wp.tile([H, CH * T], f32, tag="th4")
            nc.sync.dma_start(th4[:], th4_d[:])

            # ---- batch tiles: pair-interleaved time loop ----
            def tile_setup(b):
                c0 = b * NT
                st = {}
                fTa = fp.tile([128, 4 * NT], f32, tag="fTa")
                fTb = fp.tile([128, 3 * NT], f32, tag="fTb")
                for c in range(CH):
                    kc = KC[c]
                    dst = fTa if c < 4 else fTb
                    cc = c if c < 4 else c - 4
                    nc.sync.dma_start(
                        dst[0:kc, cc * NT : cc * NT + NT],
                        fT_d[128 * c : 128 * c + kc, c0 : c0 + NT],
                    )
                st["fTa"], st["fTb"] = fTa, fTb
                v0a = vp0.tile([128, 4 * NT], f32, tag="v0a")
                nc.gpsimd.memset(v0a[:], 0.0)
                v0b = vp0.tile([128, 3 * NT], f32, tag="v0b")
                nc.gpsimd.memset(v0b[:], 0.0)
                st["v0a"], st["v0b"] = v0a, v0b
                v4 = vp4.tile([128, (CH - NPC) * NT], f32, tag="v4")
                nc.gpsimd.memset(v4[:], -1.0)
                st["v4"] = v4
                st["v4p"] = psv4.tile([128, NPC * NT], f32, tag="v4p", name=f"v4p_{b}")
                st["v1"] = psv.tile([H, NT], f32, tag="v1", name=f"v1_{b}")
                st["v2"] = psv.tile([H, NT], f32, tag="v2", name=f"v2_{b}")
                st["v3"] = psv.tile([H, NT], f32, tag="v3", name=f"v3_{b}")
                cnt = cnp.tile([128, CW], f16, tag="cnt", name=f"cnt_{b}")
                nc.gpsimd.memset(cnt[:], 0.0)
                st["cnt"] = cnt
                st["s4_prev"] = None
                st["b"] = b
                return st

            def tile_step(st, t):
                v4 = st["v4"]
                v1, v2, v3 = st["v1"], st["v2"], st["v3"]
                v0a, v0b = st["v0a"], st["v0b"]
                # -------- encoder (two half-fronts for pipelining) --------
                nc.vector.tensor_tensor(v0a[:], v0a[:], st["fTa"][:], Alu.add)
                s0a = sp0.tile([128, 4 * NT], f16, tag="s0a")
                nc.scalar.activation(s0a[:], v0a[:], ActF.Sigmoid,
                                     bias=nk1[:], scale=KAPPA)
                nc.vector.tensor_tensor(v0b[:], v0b[:], st["fTb"][:], Alu.add)
                s0b = sp0.tile([128, 3 * NT], f16, tag="s0b")
                nc.scalar.activation(s0b[:], v0b[:], ActF.Sigmoid,
                                     bias=nk1[:], scale=KAPPA)

                # -------- layer 1 --------
                for c in range(CH):
                    kc = KC[c]
                    if c < 4:
                        rhs = s0a[0:kc, c * NT : c * NT + NT]
                    else:
                        rhs = s0b[0:kc, (c - 4) * NT : (c - 4) * NT + NT]
                    nc.tensor.matmul(v1[:], w1hi[c][:], rhs,
                                     start=(t == 0 and c == 0), stop=False,
                                     skip_group_check=True)
                    nc.tensor.matmul(v1[:], w1lo[c][:], rhs,
                                     start=False, stop=False, skip_group_check=True)
                s1 = shp.tile([H, NT], f16, tag="s1")
                nc.scalar.activation(s1[:], v1[:], ActF.Sigmoid,
                                     bias=thh[:, 0 * T + t : 0 * T + t + 1], scale=KAPPA)
                nc.tensor.matmul(v1[:], negI[:], s1[:], start=False,
                                 stop=(t == T - 1), skip_group_check=True)

                # -------- layer 2 --------
                nc.tensor.matmul(v2[:], w2hi[:], s1[:], start=(t == 0),
                                 stop=False, skip_group_check=True)
                nc.tensor.matmul(v2[:], w2lo[:], s1[:], start=False,
                                 stop=False, skip_group_check=True)
                s2 = shp.tile([H, NT], f16, tag="s2")
                nc.scalar.activation(s2[:], v2[:], ActF.Sigmoid,
                                     bias=thh[:, 1 * T + t : 1 * T + t + 1], scale=KAPPA)
                nc.tensor.matmul(v2[:], negI[:], s2[:], start=False,
                                 stop=(t == T - 1), skip_group_check=True)

                # -------- layer 3 --------
                nc.tensor.matmul(v3[:], w3hi[:], s2[:], start=(t == 0),
                                 stop=False, skip_group_check=True)
                nc.tensor.matmul(v3[:], w3lo[:], s2[:], start=False,
                                 stop=False, skip_group_check=True)
                s3 = shp.tile([H, NT], f16, tag="s3")
                nc.scalar.activation(s3[:], v3[:], ActF.Sigmoid,
                                     bias=thh[:, 2 * T + t : 2 * T + t + 1], scale=KAPPA)
                nc.tensor.matmul(v3[:], negI[:], s3[:], start=False,
                                 stop=(t == T - 1), skip_group_check=True)

                # -------- layer 4 --------
                s4 = sp4.tile([128, CW], f16, tag="s4")
                s4_prev = st["s4_prev"]
                v4p = st["v4p"]
                for m in range(NPC):
                    # PSUM-resident chunk: matmul-only accumulation
                    mc = KC[m]
                    dst = v4p[0:mc, m * NT : m * NT + NT]
                    nc.tensor.matmul(dst, w4hi[m][:], s3[:],
                                     start=(t == 0), stop=False, skip_group_check=True)
                    nc.tensor.matmul(dst, w4lo[m][:], s3[:],
                                     start=False, stop=False, skip_group_check=True)
                    if s4_prev is not None:
                        nc.tensor.matmul(
                            dst, negI[0:mc, 0:mc],
                            s4_prev[0:mc, m * NT : m * NT + NT],
                            start=False, stop=(t == T - 1), skip_group_check=True)
                    nc.scalar.activation(
                        s4[:, m * NT : m * NT + NT],
                        v4p[:, m * NT : m * NT + NT], ActF.Sigmoid,
                        bias=nkth4[:, m * T + t : m * T + t + 1], scale=KAPPA)
                for m in range(NPC, CH):
                    mc = KC[m]
                    p4 = psp4.tile([128, NT], f32, tag="p4")
                    nc.tensor.matmul(p4[0:mc, :], w4hi[m][:], s3[:],
                                     start=True, stop=False, skip_group_check=True)
                    nc.tensor.matmul(p4[0:mc, :], w4lo[m][:], s3[:],
                                     start=False, stop=False, skip_group_check=True)
                    if s4_prev is not None:
                        nc.tensor.matmul(
                            p4[0:mc, :], negI[0:mc, 0:mc],
                            s4_prev[0:mc, m * NT : m * NT + NT],
                            start=False, stop=True, skip_group_check=True)
                    vm = m - NPC
                    v4m = v4[:, vm * NT : vm * NT + NT]
                    nc.vector.scalar_tensor_tensor(
                        v4m, v4m, b4c[:, m : m + 1], p4[:], Alu.add, Alu.add)
                nc.scalar.activation(s4[:, NPC * NT :], v4[:], ActF.Sigmoid,
                                     bias=zb[:], scale=KAPPA)
                st["s4_prev"] = s4

                # encoder reset, deferred to overlap sigmoid latency
                nc.vector.tensor_tensor(v0a[:], v0a[:], s0a[:], Alu.subtract)
                nc.vector.tensor_tensor(v0b[:], v0b[:], s0b[:], Alu.subtract)

                # -------- spike count (flat fp16 accumulator, exact) --------
                cnt = st["cnt"]
                nc.vector.tensor_tensor(cnt[:], cnt[:], s4[:], Alu.add)

            def tile_finish(st):
                total = st["cnt"]
                c0 = st["b"] * NT
                for c in range(CH):
                    kc = KC[c]
                    out = outp.tile([128, NT], f32, tag="out")
                    nc.vector.tensor_scalar(
                        out[:], total[:, c * NT : c * NT + NT],
                        1.0 / 16.0, None, Alu.mult)
                    nc.sync.dma_start(
                        out_d[128 * c : 128 * c + kc, c0 : c0 + NT],
                        out[0:kc, :],
                    )

            for b in range(NTILES):
                st = tile_setup(b)
                for t in range(T):
                    tile_step(st, t)
                tile_finish(st)

    nc.compile()
    return nc


def _host_prep(inputs):
    f32 = np.float32
    f16 = np.float16
    features = np.asarray(inputs["features"], f32)
    fT = np.ascontiguousarray(features.T)  # [784, 16384]

    def split(wT):
        hi = wT.astype(f16)
        lo = (wT - hi.astype(f32)).astype(f16)
        return np.ascontiguousarray(hi), np.ascontiguousarray(lo)

    w1hi, w1lo = split(np.asarray(inputs["W1"], f32).T)  # [784, 128]
    w2hi, w2lo = split(np.asarray(inputs["W2"], f32).T)  # [128, 128]
    w3hi, w3lo = split(np.asarray(inputs["W3"], f32).T)
    w4hi, w4lo = split(np.asarray(inputs["W4"], f32).T)  # [128, 784]
    negI = (-np.eye(H)).astype(f16)

    # threshold tables: th[o, t] = 1 - cumsum(b)[t]  (fp32 iterative cumsum)
    def cum_th(b):
        b = np.asarray(b, f32)
        c = np.zeros_like(b)
        th = np.empty((b.shape[0], T), f32)
        for t in range(T):
            c = (c + b).astype(f32)
            th[:, t] = (np.float32(1.0) - c).astype(f32)
        return th

    thh = np.concatenate(
        [cum_th(inputs["b1"]), cum_th(inputs["b2"]), cum_th(inputs["b3"])], axis=1
    )  # [128, 48]
    th4_full = cum_th(inputs["b4"])  # [784, 16]
    th4 = np.zeros((H, CH * T), f32)
    for c in range(CH):
        kc = KC[c]
        th4[0:kc, c * T : (c + 1) * T] = th4_full[128 * c : 128 * c + kc, :]

    shared = {
        "w1hi": w1hi, "w1lo": w1lo, "w2hi": w2hi, "w2lo": w2lo,
        "w3hi": w3hi, "w3lo": w3lo, "w4hi": w4hi, "w4lo": w4lo,
        "negI": negI, "thh": np.ascontiguousarray(thh),
        "th4": np.ascontiguousarray(th4),
    }
    in_maps = []
    for i in range(NCORES):
        m = dict(shared)
        m["fT"] = np.ascontiguousarray(fT[:, i * BC : (i + 1) * BC])
        in_maps.append(m)
    return in_maps


def kernel(**inputs):
    global LAST_RESULT
    if os.environ.get("BASS_TRACE"):
        _install_ntff_shim()
    from concourse.bass_utils import run_bass_kernel_spmd

    if "nc" not in _CACHE:
        _CACHE["nc"] = _build()
    nc = _CACHE["nc"]

    in_maps = _host_prep(inputs)
    kwargs = {}
    if os.environ.get("BASS_TRACE"):
        kwargs["tmpdir"] = os.environ.get("BASS_TRACE_DIR") or None
    try:
        res = run_bass_kernel_spmd(nc, in_maps, core_ids=list(range(NCORES)), **kwargs)
    except Exception:
        # transient device faults (e.g. NRT_EXEC_UNIT_UNRECOVERABLE) usually
        # clear on retry
        import time

        time.sleep(2)
        res = run_bass_kernel_spmd(nc, in_maps, core_ids=list(range(NCORES)), **kwargs)
    LAST_RESULT = res

    outT = np.concatenate([res.results[i]["outT"] for i in range(NCORES)], axis=1)
    return np.ascontiguousarray(outT.T).astype(np.float32)


# revision 18
# speedup vs baseline: 1.0631x; 1.0631x over previous
"""Trainium2 Bass kernel for nn_AE_spikes (spiking autoencoder, 16-step scan).

Data-parallel over 8 NeuronCores: batch 16384 -> 2048 rows/core.

Layout: feature-major ("transposed") on device. All [784]-row tensors are
stored as [128 partitions, 7*NT] with chunk c (feature rows 128c..128c+kc)
occupying columns [c*NT, (c+1)*NT). Batch tile NT columns.

Engine plan:
  PE    : all matmuls as fp16 hi/lo weight splits (exact products for binary
          spikes), hidden membranes v1..v3 PSUM-resident, accumulated by
          matmul only; spike resets via -I identity matmuls.
  DVE   : threshold compares (is_ge, exact fp32 semantics), v0/v4 updates,
          spike-count pair tree in fp16.
  GPSIMD: encoder integrate (v0 += f).
"""

import os
import sys

import numpy as np

if "/opt/trn_rl_repo" not in sys.path:
    sys.path.insert(0, "/opt/trn_rl_repo")

B = 16384
IN = 784
H = 128
T = 16
NCORES = 8
BC = B // NCORES          # 2048 batch rows per core
NT = 512                  # batch-tile columns
NTILES = BC // NT         # 4
CH = 7                    # feature chunks: 6*128 + 16
KC = [128] * 6 + [16]
CW = CH * NT              # concatenated width, 3584

LAST_RESULT = None
_CACHE = {}


def _install_ntff_shim():
    """Make run_bass_kernel_spmd(trace=True) work in this container."""
    import types

    try:
        from antenv.axon_hooks import get_axon_ntff_profile_hook  # noqa: F401
        return
    except ImportError:
        pass
    try:
        import antenv
        from trn_agent_boot.trn_boot import _ntff_profile_via_ctypes
    except ImportError:
        return
    mod = types.ModuleType("antenv.axon_hooks")
    mod._hook = _ntff_profile_via_ctypes("/opt/axon/libaxon_pjrt.so")
    mod.set_axon_ntff_profile_hook = lambda h: setattr(mod, "_hook", h)
    mod.get_axon_ntff_profile_hook = lambda: mod._hook
    sys.modules["antenv.axon_hooks"] = mod
    antenv.axon_hooks = mod


def _build():
    import concourse.tile as tile
    from concourse import bacc, mybir
    from contextlib import ExitStack

    f32 = mybir.dt.float32
    f16 = mybir.dt.float16
    Alu = mybir.AluOpType

    nc = bacc.Bacc("TRN2", target_bir_lowering=False, debug=False)

    fT_d = nc.dram_tensor("fT", [IN, BC], f32, kind="ExternalInput").ap()
    w1hi_d = nc.dram_tensor("w1hi", [IN, H], f16, kind="ExternalInput").ap()
    w1lo_d = nc.dram_tensor("w1lo", [IN, H], f16, kind="ExternalInput").ap()
    w2hi_d = nc.dram_tensor("w2hi", [H, H], f16, kind="ExternalInput").ap()
    w2lo_d = nc.dram_tensor("w2lo", [H, H], f16, kind="ExternalInput").ap()
    w3hi_d = nc.dram_tensor("w3hi", [H, H], f16, kind="ExternalInput").ap()
    w3lo_d = nc.dram_tensor("w3lo", [H, H], f16, kind="ExternalInput").ap()
    w4hi_d = nc.dram_tensor("w4hi", [H, IN], f16, kind="ExternalInput").ap()
    w4lo_d = nc.dram_tensor("w4lo", [H, IN], f16, kind="ExternalInput").ap()
    negI_d = nc.dram_tensor("negI", [H, H], f16, kind="ExternalInput").ap()
    thh_d = nc.dram_tensor("thh", [H, 3 * T], f32, kind="ExternalInput").ap()
    th4_d = nc.dram_tensor("th4", [H, CH * T], f32, kind="ExternalInput").ap()
    out_d = nc.dram_tensor("outT", [IN, BC], f32, kind="ExternalOutput").ap()

    with tile.TileContext(nc) as tc:
        with ExitStack() as ctx:
            wp = ctx.enter_context(tc.tile_pool(name="weights", bufs=1))
            fp = ctx.enter_context(tc.tile_pool(name="feat", bufs=4))
            vp0 = ctx.enter_context(tc.tile_pool(name="v0p", bufs=2))
            sp0 = ctx.enter_context(tc.tile_pool(name="s0p", bufs=4))
            vp4 = ctx.enter_context(tc.tile_pool(name="v4p", bufs=2))
            sp4 = ctx.enter_context(tc.tile_pool(name="s4p", bufs=4))
            shp = ctx.enter_context(tc.tile_pool(name="shid", bufs=3))
            cnp = ctx.enter_context(tc.tile_pool(name="cnt", bufs=2))
            outp = ctx.enter_context(tc.tile_pool(name="outp", bufs=1))
            psv = ctx.enter_context(tc.tile_pool(name="psv", bufs=1, space="PSUM"))
            psv4 = ctx.enter_context(tc.tile_pool(name="psv4", bufs=1, space="PSUM"))
            psp4 = ctx.enter_context(tc.tile_pool(name="psp4", bufs=2, space="PSUM"))

            # ---- load weights / tables once ----
            w1hi = []
            w1lo = []
            for c in range(CH):
                kc = KC[c]
                thi = wp.tile([kc, H], f16, tag=f"w1hi{c}")
                nc.sync.dma_start(thi[:], w1hi_d[128 * c : 128 * c + kc, :])
                tlo = wp.tile([kc, H], f16, tag=f"w1lo{c}")
                nc.sync.dma_start(tlo[:], w1lo_d[128 * c : 128 * c + kc, :])
                w1hi.append(thi)
                w1lo.append(tlo)
            w2hi = wp.tile([H, H], f16, tag="w2hi")
            nc.sync.dma_start(w2hi[:], w2hi_d[:])
            w2lo = wp.tile([H, H], f16, tag="w2lo")
            nc.sync.dma_start(w2lo[:], w2lo_d[:])
            w3hi = wp.tile([H, H], f16, tag="w3hi")
            nc.sync.dma_start(w3hi[:], w3hi_d[:])
            w3lo = wp.tile([H, H], f16, tag="w3lo")
            nc.sync.dma_start(w3lo[:], w3lo_d[:])
            w4hi = []
            w4lo = []
            for m in range(CH):
                mc = KC[m]
                thi = wp.tile([H, mc], f16, tag=f"w4hi{m}")
                nc.sync.dma_start(thi[:], w4hi_d[:, 128 * m : 128 * m + mc])
                tlo = wp.tile([H, mc], f16, tag=f"w4lo{m}")
                nc.sync.dma_start(tlo[:], w4lo_d[:, 128 * m : 128 * m + mc])
                w4hi.append(thi)
                w4lo.append(tlo)
            negI = wp.tile([H, H], f16, tag="negI")
            nc.sync.dma_start(negI[:], negI_d[:])
            thh = wp.tile([H, 3 * T], f32, tag="thh")
            nc.sync.dma_start(thh[:], thh_d[:])
            th4 = wp.tile([H, CH * T], f32, tag="th4")
            nc.sync.dma_start(th4[:], th4_d[:])

            # ---- batch tiles: pair-interleaved time loop ----
            def tile_setup(b):
                c0 = b * NT
                st = {}
                fT = fp.tile([128, CW], f32, tag="fT")
                for c in range(CH):
                    kc = KC[c]
                    nc.sync.dma_start(
                        fT[0:kc, c * NT : c * NT + NT],
                        fT_d[128 * c : 128 * c + kc, c0 : c0 + NT],
                    )
                st["fT"] = fT
                v0 = vp0.tile([128, CW], f32, tag="v0")
                nc.gpsimd.memset(v0[:], 0.0)
                st["v0"] = v0
                v4 = vp4.tile([128, (CH - NPC) * NT], f32, tag="v4")
                nc.gpsimd.memset(v4[:], -1.0)
                st["v4"] = v4
                st["v4p"] = psv4.tile([128, NPC * NT], f32, tag="v4p", name=f"v4p_{b}")
                st["v1"] = psv.tile([H, NT], f32, tag="v1", name=f"v1_{b}")
                st["v2"] = psv.tile([H, NT], f32, tag="v2", name=f"v2_{b}")
                st["v3"] = psv.tile([H, NT], f32, tag="v3", name=f"v3_{b}")
                cnt = cnp.tile([128, CW], f16, tag="cnt", name=f"cnt_{b}")
                nc.vector.memset(cnt[:], 0.0)
                st["cnt"] = cnt
                st["s4_prev"] = None
                st["b"] = b
                return st

            def tile_step(st, t):
                v0, v4 = st["v0"], st["v4"]
                v1, v2, v3 = st["v1"], st["v2"], st["v3"]
                fT = st["fT"]
                # -------- encoder --------
                nc.vector.tensor_tensor(v0[:], v0[:], fT[:], Alu.add)
                s0 = sp0.tile([128, CW], f16, tag="s0")
                nc.scalar.activation(s0[:], v0[:], ActF.Sigmoid,
                                     bias=nk1[:], scale=KAPPA)

                # -------- layer 1 --------
                for c in range(CH):
                    kc = KC[c]
                    rhs = s0[0:kc, c * NT : c * NT + NT]
                    nc.tensor.matmul(v1[:], w1hi[c][:], rhs,
                                     start=(t == 0 and c == 0), stop=False,
                                     skip_group_check=True)
                    nc.tensor.matmul(v1[:], w1lo[c][:], rhs,
                                     start=False, stop=False, skip_group_check=True)
                s1 = shp.tile([H, NT], f16, tag="s1")
                nc.scalar.activation(s1[:], v1[:], ActF.Sigmoid,
                                     bias=thh[:, 0 * T + t : 0 * T + t + 1], scale=KAPPA)
                nc.tensor.matmul(v1[:], negI[:], s1[:], start=False,
                                 stop=(t == T - 1), skip_group_check=True)

                # -------- layer 2 --------
                nc.tensor.matmul(v2[:], w2hi[:], s1[:], start=(t == 0),
                                 stop=False, skip_group_check=True)
                nc.tensor.matmul(v2[:], w2lo[:], s1[:], start=False,
                                 stop=False, skip_group_check=True)
                s2 = shp.tile([H, NT], f16, tag="s2")
                nc.scalar.activation(s2[:], v2[:], ActF.Sigmoid,
                                     bias=thh[:, 1 * T + t : 1 * T + t + 1], scale=KAPPA)
                nc.tensor.matmul(v2[:], negI[:], s2[:], start=False,
                                 stop=(t == T - 1), skip_group_check=True)

                # -------- layer 3 --------
                nc.tensor.matmul(v3[:], w3hi[:], s2[:], start=(t == 0),
                                 stop=False, skip_group_check=True)
                nc.tensor.matmul(v3[:], w3lo[:], s2[:], start=False,
                                 stop=False, skip_group_check=True)
                s3 = shp.tile([H, NT], f16, tag="s3")
                nc.scalar.activation(s3[:], v3[:], ActF.Sigmoid,
                                     bias=thh[:, 2 * T + t : 2 * T + t + 1], scale=KAPPA)
                nc.tensor.matmul(v3[:], negI[:], s3[:], start=False,
                                 stop=(t == T - 1), skip_group_check=True)

                # -------- layer 4 --------
                s4 = sp4.tile([128, CW], f16, tag="s4")
                s4_prev = st["s4_prev"]
                v4p = st["v4p"]
                for m in range(NPC):
                    # PSUM-resident chunk: matmul-only accumulation
                    mc = KC[m]
                    dst = v4p[0:mc, m * NT : m * NT + NT]
                    nc.tensor.matmul(dst, w4hi[m][:], s3[:],
                                     start=(t == 0), stop=False, skip_group_check=True)
                    nc.tensor.matmul(dst, w4lo[m][:], s3[:],
                                     start=False, stop=False, skip_group_check=True)
                    if s4_prev is not None:
                        nc.tensor.matmul(
                            dst, negI[0:mc, 0:mc],
                            s4_prev[0:mc, m * NT : m * NT + NT],
                            start=False, stop=(t == T - 1), skip_group_check=True)
                    nc.scalar.activation(
                        s4[:, m * NT : m * NT + NT],
                        v4p[:, m * NT : m * NT + NT], ActF.Sigmoid,
                        bias=nkth4[:, m * T + t : m * T + t + 1], scale=KAPPA)
                for m in range(NPC, CH):
                    mc = KC[m]
                    p4 = psp4.tile([128, NT], f32, tag="p4")
                    nc.tensor.matmul(p4[0:mc, :], w4hi[m][:], s3[:],
                                     start=True, stop=False, skip_group_check=True)
                    nc.tensor.matmul(p4[0:mc, :], w4lo[m][:], s3[:],
                                     start=False, stop=False, skip_group_check=True)
                    if s4_prev is not None:
                        nc.tensor.matmul(
                            p4[0:mc, :], negI[0:mc, 0:mc],
                            s4_prev[0:mc, m * NT : m * NT + NT],
                            start=False, stop=True, skip_group_check=True)
                    vm = m - NPC
                    v4m = v4[:, vm * NT : vm * NT + NT]
                    nc.vector.scalar_tensor_tensor(
                        v4m, v4m, b4c[:, m : m + 1], p4[:], Alu.add, Alu.add)
                nc.scalar.activation(s4[:, NPC * NT :], v4[:], ActF.Sigmoid,
                                     bias=zb[:], scale=KAPPA)
                st["s4_prev"] = s4

                # encoder reset, deferred to overlap sigmoid latency
                nc.vector.tensor_tensor(v0[:], v0[:], s0[:], Alu.subtract)

                # -------- spike count (flat fp16 accumulator, exact) --------
                cnt = st["cnt"]
                nc.vector.tensor_tensor(cnt[:], cnt[:], s4[:], Alu.add)

            def tile_finish(st):
                total = st["cnt"]
                c0 = st["b"] * NT
                for c in range(CH):
                    kc = KC[c]
                    out = outp.tile([128, NT], f32, tag="out")
                    nc.vector.tensor_scalar(
                        out[:], total[:, c * NT : c * NT + NT],
                        1.0 / 16.0, None, Alu.mult)
                    nc.sync.dma_start(
                        out_d[128 * c : 128 * c + kc, c0 : c0 + NT],
                        out[0:kc, :],
                    )

            for b in range(NTILES):
                st = tile_setup(b)
                for t in range(T):
                    tile_step(st, t)
                tile_finish(st)

    nc.compile()
    return nc


def _host_prep(inputs):
    f32 = np.float32
    f16 = np.float16
    features = np.asarray(inputs["features"], f32)
    fT = np.ascontiguousarray(features.T)  # [784, 16384]

    def split(wT):
        hi = wT.astype(f16)
        lo = (wT - hi.astype(f32)).astype(f16)
        return np.ascontiguousarray(hi), np.ascontiguousarray(lo)

    w1hi, w1lo = split(np.asarray(inputs["W1"], f32).T)  # [784, 128]
    w2hi, w2lo = split(np.asarray(inputs["W2"], f32).T)  # [128, 128]
    w3hi, w3lo = split(np.asarray(inputs["W3"], f32).T)
    w4hi, w4lo = split(np.asarray(inputs["W4"], f32).T)  # [128, 784]
    negI = (-np.eye(H)).astype(f16)

    # threshold tables: th[o, t] = 1 - cumsum(b)[t]  (fp32 iterative cumsum)
    def cum_th(b):
        b = np.asarray(b, f32)
        c = np.zeros_like(b)
        th = np.empty((b.shape[0], T), f32)
        for t in range(T):
            c = (c + b).astype(f32)
            th[:, t] = (np.float32(1.0) - c).astype(f32)
        return th

    thh = np.concatenate(
        [cum_th(inputs["b1"]), cum_th(inputs["b2"]), cum_th(inputs["b3"])], axis=1
    )  # [128, 48]
    th4_full = cum_th(inputs["b4"])  # [784, 16]
    th4 = np.zeros((H, CH * T), f32)
    for c in range(CH):
        kc = KC[c]
        th4[0:kc, c * T : (c + 1) * T] = th4_full[128 * c : 128 * c + kc, :]

    shared = {
        "w1hi": w1hi, "w1lo": w1lo, "w2hi": w2hi, "w2lo": w2lo,
        "w3hi": w3hi, "w3lo": w3lo, "w4hi": w4hi, "w4lo": w4lo,
        "negI": negI, "thh": np.ascontiguousarray(thh),
        "th4": np.ascontiguousarray(th4),
    }
    in_maps = []
    for i in range(NCORES):
        m = dict(shared)
        m["fT"] = np.ascontiguousarray(fT[:, i * BC : (i + 1) * BC])
        in_maps.append(m)
    return in_maps


def kernel(**inputs):
    global LAST_RESULT
    if os.environ.get("BASS_TRACE"):
        _install_ntff_shim()
    from concourse.bass_utils import run_bass_kernel_spmd

    if "nc" not in _CACHE:
        _CACHE["nc"] = _build()
    nc = _CACHE["nc"]

    in_maps = _host_prep(inputs)
    kwargs = {}
    if os.environ.get("BASS_TRACE"):
        kwargs["tmpdir"] = os.environ.get("BASS_TRACE_DIR") or None
    try:
        res = run_bass_kernel_spmd(nc, in_maps, core_ids=list(range(NCORES)), **kwargs)
    except Exception:
        # transient device faults (e.g. NRT_EXEC_UNIT_UNRECOVERABLE) usually
        # clear on retry
        import time

        time.sleep(2)
        res = run_bass_kernel_spmd(nc, in_maps, core_ids=list(range(NCORES)), **kwargs)
    LAST_RESULT = res

    outT = np.concatenate([res.results[i]["outT"] for i in range(NCORES)], axis=1)
    return np.ascontiguousarray(outT.T).astype(np.float32)
